# revision 6
# baseline (speedup 1.0000x reference)
"""Trainium2 Bass kernel for nn_NMP_IntNet (gnn_message_passing).

Full-input contract: kernel(**inputs) takes the unsharded inputs from
setup_inputs() and returns the full [32, 12] output. Internally shards
batch B=32 over 8 NeuronCores (4 graphs per core), runs one SPMD Bass
program via run_bass_kernel_spmd, and gathers.

Math per round t (2 rounds), per graph:
  pair(v,w) = [h_v, h_w, e_vw]                       # rows r = 64*v + w
  x1 = relu(W1e^T e^T + Y(v) + Z(w))                 # Y = h W1v + b1, Z = h W1w
  x2 = relu(W2^T x1 + b2); x3 = relu(W3^T x2 + b3)
  s_v = sum_w g[v,w] * x3[:, (v,w)]                  # gated sum
  m = W4^T s + b4 * G_v (G_v = sum_w g)              # fold W4 after the sum
  h' = relu-MLP_u([h; m])
Readout: MLP_r(sum_v h_final) -> [12] per graph.

Layout: activations are channel-major [C (partitions), rows (free)].
Y/Z enter the all-pairs tensor through a constant one-hot matrix OHIW
([v-indicator; w-indicator] rows) on the TensorEngine. Tensors feeding
matmuls are float32r (1 cycle/row at N>=256 vs 4 for fp32); the BIR
verifier requires producers to write rounded f32r, so those tiles are
declared float32r natively.
"""

import os

os.environ.setdefault("MYCRO_LOCAL_CACHE", "1")

import numpy as np

import concourse.bass as bass
import concourse.mybir as mybir
import concourse.tile as tile
from concourse import bacc

F32 = mybir.dt.float32
F32R = mybir.dt.float32r
AX = mybir.AxisListType
ALU = mybir.AluOpType
AF = mybir.ActivationFunctionType

B, N, DV, DE = 32, 64, 64, 16
NCORES = 8
GPC = B // NCORES          # graphs per core
NODES = GPC * N            # nodes per core (256)
RPG = N * N                # all-pairs rows per graph (4096)
CH = 512                   # chunk rows
CPG = RPG // CH            # chunks per graph (8)
D_T = [64, 96, 96]         # h dim entering round t (and final)
OUT_M = 96
OUT_U = 96
LT = 12
NB = 21                    # bias-pack columns


def _bcast(ap, parts):
    """Broadcast a 1-D DRAM AP across `parts` partitions (stride-0)."""
    return bass.AP(tensor=ap.tensor, offset=ap.offset, ap=[[0, parts], *ap.ap])


def _build():
    nc = bacc.Bacc(
        "TRN2",
        target_bir_lowering=False,
        debug=False,
        enable_asserts=False,
        num_devices=NCORES,
    )

    din = {}
    ddt = {}

    def dram_in(name, shape, dt=F32R):
        din[name] = nc.dram_tensor(name, list(shape), dt, kind="ExternalInput").ap()
        ddt[name] = dt
        return din[name]

    eT = dram_in("eT", (GPC, DE, RPG))
    gam = dram_in("gam", (GPC, RPG), F32)
    Grow = dram_in("Grow", (1, NODES))
    h0T = dram_in("h0T", (D_T[0] + 1, NODES))
    OHIW = dram_in("OHIW", (128, RPG))
    for t in range(2):
        d = D_T[t]
        dram_in(f"mW1e_{t}", (DE, 128))
        dram_in(f"mWy_{t}", (d + 1, 128))
        dram_in(f"mWz_{t}", (d, 128))
        dram_in(f"mW2_{t}", (128, 256))
        dram_in(f"mW3a_{t}", (128, 128))
        dram_in(f"mW3b_{t}", (128, 128))
        dram_in(f"mW4_{t}", (128, OUT_M))
        dram_in(f"mb4_{t}", (1, OUT_M))
        dram_in(f"uW1h_{t}", (d, 128))
        dram_in(f"uW1m_{t}", (OUT_M, 128))
        dram_in(f"uW2_{t}", (128, 256))
        dram_in(f"uW3a_{t}", (128, 128))
        dram_in(f"uW3b_{t}", (128, 128))
        dram_in(f"uW4_{t}", (128, OUT_U))
    dram_in("rW1", (OUT_U, 128))
    dram_in("rW2", (128, 256))
    dram_in("rW3a", (128, 128))
    dram_in("rW3b", (128, 128))
    dram_in("rW4", (128, LT))
    dram_in("biases", (128, NB), F32)

    out_dram = nc.dram_tensor("out", [GPC, LT], F32, kind="ExternalOutput").ap()

    const_names = [k for k in din if k not in ("eT", "gam")]

    with tile.TileContext(nc) as tc:
        with (
            tc.tile_pool(name="consts", bufs=1) as consts,
            tc.tile_pool(name="work", bufs=2) as work,
            tc.tile_pool(name="psum", bufs=2, space=bass.MemorySpace.PSUM) as psum,
        ):
            # ---- resident constants / weights
            sW = {}
            for k in const_names:
                shp = list(din[k].shape)
                sW[k] = consts.tile(shp, ddt[k], tag=k, name=f"s_{k}")
                nc.sync.dma_start(out=sW[k], in_=din[k])
            sB = sW["biases"]

            def bap(col, parts=128):
                return sB[0:parts, col:col + 1]

            sh = [None, None, None]
            sh[0] = sW["h0T"]
            sOHIW = sW["OHIW"]

            for t in range(2):
                d = D_T[t]
                s_all = consts.tile([128, NODES], F32R, tag=f"s_all{t}",
                                    name=f"s_all{t}")

                # ---- per-graph Y/Z (row-major, [y;z] stacked on partitions)
                YZ = []
                for g in range(GPC):
                    yp = psum.tile([64, 128], F32, tag="p_x1", bufs=2,
                                   name=f"yp{t}_{g}")
                    nc.tensor.matmul(
                        yp,
                        sh[t][0:d + 1, 64 * g:64 * (g + 1)],
                        sW[f"mWy_{t}"],
                        start=True, stop=True,
                    )
                    zp = psum.tile([64, 128], F32, tag="p_x3", bufs=2,
                                   name=f"zp{t}_{g}")
                    nc.tensor.matmul(
                        zp,
                        sh[t][0:d, 64 * g:64 * (g + 1)],
                        sW[f"mWz_{t}"],
                        start=True, stop=True,
                    )
                    yz = work.tile([128, 128], F32R, tag="yz", bufs=4,
                                   name=f"yz{t}_{g}")
                    nc.scalar.copy(yz[0:64, :], yp)
                    z_sb = work.tile([64, 128], F32R, tag="z_sb", bufs=4,
                                     name=f"zsb{t}_{g}")
                    nc.scalar.copy(z_sb, zp)
                    # partition shift 0:64 -> 64:128 via on-chip DMA
                    nc.sync.dma_start(out=yz[64:128, :], in_=z_sb)
                    YZ.append(yz)

                # ---- all-pairs chunks
                for g in range(GPC):
                    for c in range(CPG):
                        cc = g * CPG + c
                        col = slice(CH * c, CH * (c + 1))
                        eTc = work.tile([DE, CH], F32R, tag="eTc", bufs=3,
                                        name=f"eTc{t}_{cc}")
                        nc.sync.dma_start(out=eTc, in_=eT[g, :, col])
                        gmr = work.tile([128, CH], F32, tag="gmr", bufs=3,
                                        name=f"gmr{t}_{cc}")
                        nc.sync.dma_start(out=gmr, in_=_bcast(gam[g, col], 128))

                        x1p = psum.tile([128, CH], F32, tag="p_x1", bufs=2,
                                        name=f"x1p{t}_{cc}")
                        nc.tensor.matmul(x1p, sW[f"mW1e_{t}"], eTc,
                                         start=True, stop=False)
                        nc.tensor.matmul(x1p, YZ[g], sOHIW[:, col],
                                         start=False, stop=True)
                        x1 = work.tile([128, CH], F32R, tag="x1", bufs=2,
                                       name=f"x1_{t}_{cc}")
                        # DVE: relu (bias b1 already folded into Y)
                        nc.vector.tensor_scalar_max(x1, x1p, 0.0)

                        x2pa = psum.tile([128, CH], F32, tag="p_x2a", bufs=2,
                                         name=f"x2pa{t}_{cc}")
                        nc.tensor.matmul(x2pa, sW[f"mW2_{t}"][:, 0:128],
                                         x1, start=True, stop=True)
                        x2pb = psum.tile([128, CH], F32, tag="p_x2b", bufs=2,
                                         name=f"x2pb{t}_{cc}")
                        nc.tensor.matmul(x2pb, sW[f"mW2_{t}"][:, 128:256],
                                         x1, start=True, stop=True)
                        x2 = work.tile([128, 2, CH], F32R, tag="x2", bufs=2,
                                       name=f"x2_{t}_{cc}")
                        # ACT: relu(x + b) on both halves
                        nc.scalar.activation(x2[:, 0, :], x2pa, AF.Relu,
                                             bias=bap(8 * t + 0))
                        nc.scalar.activation(x2[:, 1, :], x2pb, AF.Relu,
                                             bias=bap(8 * t + 1))

                        x3p = psum.tile([128, CH], F32, tag="p_x3", bufs=2,
                                        name=f"x3p{t}_{cc}")
                        nc.tensor.matmul(x3p, sW[f"mW3a_{t}"],
                                         x2[:, 0, :], start=True, stop=False)
                        nc.tensor.matmul(x3p, sW[f"mW3b_{t}"],
                                         x2[:, 1, :], start=False, stop=True)
                        x3 = work.tile([128, CH], F32, tag="x3", bufs=2,
                                       name=f"x3_{t}_{cc}")
                        # DVE: relu(x + b3)
                        nc.vector.tensor_scalar(x3, x3p, bap(8 * t + 2), 0.0,
                                                op0=ALU.add, op1=ALU.max)

                        # GPSIMD mult + DVE windowed reduce:
                        # s[:, v] = sum_w x3 * g
                        P = work.tile([128, CH], F32, tag="P", bufs=2,
                                      name=f"P_{t}_{cc}")
                        nc.gpsimd.tensor_tensor(P, x3, gmr, op=ALU.mult)
                        with nc.allow_low_precision(reason="f32r gated-sum"):
                            nc.vector.tensor_reduce(
                                s_all[:, 8 * cc:8 * cc + 8],
                                P.rearrange("p (v w) -> p v w", w=N),
                                axis=AX.X, op=ALU.add,
                            )

                # ---- m = W4^T s + b4 * G
                mp = psum.tile([OUT_M, NODES], F32, tag="p_x2a", name=f"mp{t}")
                nc.tensor.matmul(mp, sW[f"mW4_{t}"], s_all,
                                 start=True, stop=False)
                nc.tensor.matmul(mp, sW[f"mb4_{t}"], sW["Grow"],
                                 start=False, stop=True)
                m_sb = work.tile([OUT_M, NODES], F32R, tag="m_sb", bufs=2,
                                 name=f"m_sb{t}")
                nc.scalar.copy(m_sb, mp)

                # ---- update MLP u: h' = MLP([h; m])
                u1p = psum.tile([128, NODES], F32, tag="p_x1", name=f"u1p{t}")
                nc.tensor.matmul(u1p, sW[f"uW1h_{t}"], sh[t][0:d, :],
                                 start=True, stop=False)
                nc.tensor.matmul(u1p, sW[f"uW1m_{t}"], m_sb,
                                 start=False, stop=True)
                u1 = work.tile([128, NODES], F32R, tag="u1", bufs=2,
                               name=f"u1_{t}")
                nc.scalar.activation(u1, u1p, AF.Relu, bias=bap(8 * t + 3))

                u2pa = psum.tile([128, NODES], F32, tag="p_x2a", name=f"u2pa{t}")
                nc.tensor.matmul(u2pa, sW[f"uW2_{t}"][:, 0:128], u1,
                                 start=True, stop=True)
                u2pb = psum.tile([128, NODES], F32, tag="p_x2b", name=f"u2pb{t}")
                nc.tensor.matmul(u2pb, sW[f"uW2_{t}"][:, 128:256], u1,
                                 start=True, stop=True)
                u2 = work.tile([128, 2, NODES], F32R, tag="u2", bufs=2,
                               name=f"u2_{t}")
                nc.scalar.activation(u2[:, 0, :], u2pa, AF.Relu,
                                     bias=bap(8 * t + 4))
                nc.vector.tensor_scalar(u2[:, 1, :], u2pb, bap(8 * t + 5), 0.0,
                                        op0=ALU.add, op1=ALU.max)

                u3p = psum.tile([128, NODES], F32, tag="p_x3", name=f"u3p{t}")
                nc.tensor.matmul(u3p, sW[f"uW3a_{t}"], u2[:, 0, :],
                                 start=True, stop=False)
                nc.tensor.matmul(u3p, sW[f"uW3b_{t}"], u2[:, 1, :],
                                 start=False, stop=True)
                u3 = work.tile([128, NODES], F32R, tag="u3", bufs=2,
                               name=f"u3_{t}")
                nc.scalar.activation(u3, u3p, AF.Relu, bias=bap(8 * t + 6))

                u4p = psum.tile([OUT_U, NODES], F32, tag="p_x1", name=f"u4p{t}")
                nc.tensor.matmul(u4p, sW[f"uW4_{t}"], u3,
                                 start=True, stop=True)
                hn = consts.tile([D_T[t + 1] + 1, NODES], F32R,
                                 tag=f"h{t + 1}", name=f"h{t + 1}")
                nc.vector.tensor_scalar_add(hn[0:OUT_U, :], u4p,
                                            bap(8 * t + 7, OUT_U))
                nc.vector.memset(hn[OUT_U:OUT_U + 1, :].bitcast(F32), 1.0)
                sh[t + 1] = hn

            # ---- readout
            hF = sh[2]
            hsum = work.tile([OUT_U, GPC], F32R, tag="hsum", bufs=1,
                             name="hsum")
            with nc.allow_low_precision(reason="f32r readout sum"):
                nc.vector.tensor_reduce(
                    hsum,
                    hF[0:OUT_U, :].rearrange("c (g n) -> c g n", g=GPC),
                    axis=AX.X, op=ALU.add,
                )
            r1p = psum.tile([128, GPC], F32, tag="p_x1", name="r1p")
            nc.tensor.matmul(r1p, sW["rW1"], hsum, start=True, stop=True)
            r1 = work.tile([128, GPC], F32R, tag="r1", bufs=1, name="r1")
            nc.vector.tensor_scalar(r1, r1p, bap(16), 0.0, op0=ALU.add,
                                    op1=ALU.max)
            r2pa = psum.tile([128, GPC], F32, tag="p_x2a", name="r2pa")
            nc.tensor.matmul(r2pa, sW["rW2"][:, 0:128], r1,
                             start=True, stop=True)
            r2pb = psum.tile([128, GPC], F32, tag="p_x2b", name="r2pb")
            nc.tensor.matmul(r2pb, sW["rW2"][:, 128:256], r1,
                             start=True, stop=True)
            r2 = work.tile([128, 2, GPC], F32R, tag="r2", bufs=1, name="r2")
            nc.vector.tensor_scalar(r2[:, 0, :], r2pa, bap(17), 0.0,
                                    op0=ALU.add, op1=ALU.max)
            nc.vector.tensor_scalar(r2[:, 1, :], r2pb, bap(18), 0.0,
                                    op0=ALU.add, op1=ALU.max)
            r3p = psum.tile([128, GPC], F32, tag="p_x3", name="r3p")
            nc.tensor.matmul(r3p, sW["rW3a"], r2[:, 0, :],
                             start=True, stop=False)
            nc.tensor.matmul(r3p, sW["rW3b"], r2[:, 1, :],
                             start=False, stop=True)
            r3 = work.tile([128, GPC], F32R, tag="r3", bufs=1, name="r3")
            nc.vector.tensor_scalar(r3, r3p, bap(19), 0.0, op0=ALU.add,
                                    op1=ALU.max)
            r4p = psum.tile([LT, GPC], F32, tag="p_x1", name="r4p")
            nc.tensor.matmul(r4p, sW["rW4"], r3, start=True, stop=True)
            out_sb = work.tile([LT, GPC], F32, tag="out_sb", bufs=1,
                               name="out_sb")
            nc.vector.tensor_scalar_add(out_sb, r4p, bap(20, LT))
            nc.sync.dma_start(out=out_dram.rearrange("g l -> l g"), in_=out_sb)

    nc.compile()
    return nc


_NC = None


def _get_nc():
    global _NC
    if _NC is None:
        _NC = _build()
    return _NC


def _np(x):
    return np.ascontiguousarray(np.asarray(x, dtype=np.float32))


def _prep_shared(m_params, u_params, r_params):
    """Host-side weight repacking (shared by all cores)."""
    sh = {}
    r_idx = np.arange(RPG)
    ohiw = np.zeros((128, RPG), np.float32)
    ohiw[r_idx // N, r_idx] = 1.0
    ohiw[64 + r_idx % N, r_idx] = 1.0
    sh["OHIW"] = ohiw

    biases = np.zeros((128, NB), np.float32)
    for t in range(2):
        d = D_T[t]
        (W1, b1), (W2, b2), (W3, b3), (W4, b4) = [(_np(w), _np(b))
                                                  for w, b in m_params[t]]
        sh[f"mW1e_{t}"] = _np(W1[2 * d:2 * d + DE])
        sh[f"mWy_{t}"] = np.vstack([W1[:d], b1[None, :]])
        sh[f"mWz_{t}"] = _np(W1[d:2 * d])
        sh[f"mW2_{t}"] = W2
        sh[f"mW3a_{t}"] = _np(W3[:128])
        sh[f"mW3b_{t}"] = _np(W3[128:])
        sh[f"mW4_{t}"] = W4
        sh[f"mb4_{t}"] = b4[None, :]
        biases[:, 8 * t + 0] = b2[:128]
        biases[:, 8 * t + 1] = b2[128:]
        biases[:, 8 * t + 2] = b3
        (U1, bu1), (U2, bu2), (U3, bu3), (U4, bu4) = [(_np(w), _np(b))
                                                      for w, b in u_params[t]]
        sh[f"uW1h_{t}"] = _np(U1[:d])
        sh[f"uW1m_{t}"] = _np(U1[d:])
        sh[f"uW2_{t}"] = U2
        sh[f"uW3a_{t}"] = _np(U3[:128])
        sh[f"uW3b_{t}"] = _np(U3[128:])
        sh[f"uW4_{t}"] = U4
        biases[:, 8 * t + 3] = bu1
        biases[:, 8 * t + 4] = bu2[:128]
        biases[:, 8 * t + 5] = bu2[128:]
        biases[:, 8 * t + 6] = bu3
        biases[:OUT_U, 8 * t + 7] = bu4
    (R1, br1), (R2, br2), (R3, br3), (R4, br4) = [(_np(w), _np(b))
                                                  for w, b in r_params]
    sh["rW1"] = R1
    sh["rW2"] = R2
    sh["rW3a"] = _np(R3[:128])
    sh["rW3b"] = _np(R3[128:])
    sh["rW4"] = R4
    biases[:, 16] = br1
    biases[:, 17] = br2[:128]
    biases[:, 18] = br2[128:]
    biases[:, 19] = br3
    biases[:LT, 20] = br4
    sh["biases"] = biases
    return sh


def make_in_maps(g, h_in, e, m_params, u_params, r_params):
    g = _np(g)
    h_in = _np(h_in)
    e = _np(e)
    shared = _prep_shared(m_params, u_params, r_params)
    in_maps = []
    ones = np.ones((1, NODES), np.float32)
    for cc in range(NCORES):
        sl = slice(GPC * cc, GPC * (cc + 1))
        ec = e[sl]                                   # [GPC, N, N, DE]
        hc = h_in[sl]                                # [GPC, N, DV]
        gc = g[sl]                                   # [GPC, N, N]
        m = dict(shared)
        m["eT"] = np.ascontiguousarray(
            ec.transpose(0, 3, 1, 2).reshape(GPC, DE, RPG))
        m["gam"] = np.ascontiguousarray(gc.reshape(GPC, RPG))
        m["Grow"] = np.ascontiguousarray(gc.sum(-1).reshape(1, NODES))
        m["h0T"] = np.vstack(
            [hc.transpose(2, 0, 1).reshape(DV, NODES), ones])
        in_maps.append(m)
    return in_maps


def _install_ntff_shim():
    """The trimmed image lacks antenv.axon_hooks; recreate it so
    run_bass_kernel_spmd(trace=True) can capture NTFF profiles."""
    import sys
    import types

    try:
        from antenv.axon_hooks import get_axon_ntff_profile_hook  # noqa: F401
        return
    except ImportError:
        pass
    import antenv
    from trn_agent_boot.trn_boot import _ntff_profile_via_ctypes

    mod = types.ModuleType("antenv.axon_hooks")
    state = {"hook": _ntff_profile_via_ctypes("/opt/axon/libaxon_pjrt.so")}
    mod.set_axon_ntff_profile_hook = lambda h: state.update(hook=h)
    mod.get_axon_ntff_profile_hook = lambda: state["hook"]
    sys.modules["antenv.axon_hooks"] = mod
    antenv.axon_hooks = mod


def run(g, h_in, e, m_params, u_params, r_params, trace=False, **kw):
    from concourse.bass_utils import run_bass_kernel_spmd

    if trace:
        _install_ntff_shim()
    nc = _get_nc()
    in_maps = make_in_maps(g, h_in, e, m_params, u_params, r_params)
    res = run_bass_kernel_spmd(nc, in_maps, core_ids=list(range(NCORES)),
                               trace=trace, **kw)
    out = np.concatenate([r["out"] for r in res.results], axis=0)
    return out.astype(np.float32), res


def kernel(g, h_in, e, m_params, u_params, r_params):
    out, _ = run(g, h_in, e, m_params, u_params, r_params, trace=False)
    return out
